# revision 3
# baseline (speedup 1.0000x reference)
"""Causal self-attention Trainium2 kernel.

Problem: B=2, L=2048, D=1024, 16 heads (hd=64), fp32.

Sharding (8 cores): core = (batch b in {0,1}) x (head-group g in {0..3} of 4
heads). Each core:
  - reads x[b]^T  [1024, 2048]
  - QKV projection for its 4 heads (fp32r matmuls, full PE rate at N>=256)
  - causal attention in transposed layout:
      S^T[k, q] = K^T(lhsT) x Q^T(rhs), two heads row-packed per matmul (K=64)
      P^T = exp(S^T)  (ACT), causal mask via 0/1 mask multiply (DVE)
      O^T[d, q] accumulated as [V | ones]^T(lhsT) x P^T(rhs) -> row 64 = rowsum
      normalize: reciprocal (DVE) -> partition_broadcast (GPSIMD) -> mul (DVE)
  - output projection partial: OUT[tok, :] = O^T-chunks(lhsT) x Wo^T(rhs)
Host: sums the 4 head-group partials per batch, adds out_b.

All matmul operands are typed float32r (TF32-like, ~1.5e-4 rel err/matmul,
full 1 cycle/row PE rate at N>=256 vs 4 cycles/row for fp32).
"""
import os
import numpy as np

import concourse.bass as bass
import concourse.mybir as mybir
import concourse.tile as tile
from concourse import bacc
from concourse.bass_utils import run_bass_kernel_spmd

F32 = mybir.dt.float32
F32R = mybir.dt.float32r
AF = mybir.ActivationFunctionType

D_MODEL = 1024
N_HEADS = 16
HD = 64
B = 2
L = 2048                      # tokens per batch
HPC = 4                       # heads per core
DG = HPC * HD                 # 256 dims per core's head group
QB = 512                      # q-block width
N_QB = L // QB                # 4
N_KC = L // 128               # 16 k-chunks of 128 tokens
N_DC = D_MODEL // 128         # 8 d_model chunks
N_TT = L // 128               # 16 token tiles


def _build():
    nc = bacc.Bacc("TRN2", target_bir_lowering=False)

    xt = nc.dram_tensor("xt", [D_MODEL, L], F32R, kind="ExternalInput")
    wq = nc.dram_tensor("wq", [D_MODEL, DG], F32R, kind="ExternalInput")
    wk = nc.dram_tensor("wk", [D_MODEL, DG], F32R, kind="ExternalInput")
    wv = nc.dram_tensor("wv", [D_MODEL, DG], F32R, kind="ExternalInput")
    wo = nc.dram_tensor("wo", [DG, D_MODEL], F32R, kind="ExternalInput")
    bq = nc.dram_tensor("bq", [128, 2], F32, kind="ExternalInput")
    bk = nc.dram_tensor("bk", [128, 2], F32, kind="ExternalInput")
    bv = nc.dram_tensor("bv", [1, DG], F32R, kind="ExternalInput")
    # masks[p, i, c, q] = 1 iff -256*i + q - 128*c - p >= 0  (i in {0,1})
    masks = nc.dram_tensor("masks", [128, 2, 2, QB], F32R, kind="ExternalInput")
    out = nc.dram_tensor("out", [L, D_MODEL], F32, kind="ExternalOutput")

    with tile.TileContext(nc) as tc:
        with (
            tc.tile_pool(name="cst", bufs=1) as cst,
            tc.tile_pool(name="xtp", bufs=2) as xtp,
            tc.tile_pool(name="ptp", bufs=2) as ptp,
            tc.tile_pool(name="nrm", bufs=2) as nrm,
            tc.tile_pool(name="osb", bufs=2) as osb,
            tc.tile_pool(name="ps_st", bufs=1, space="PSUM") as ps_st,
            tc.tile_pool(name="ps_ot", bufs=1, space="PSUM") as ps_ot,
            tc.tile_pool(name="ps_mm", bufs=2, space="PSUM") as ps_mm,
        ):
            # ---- constants / weights ----
            wq_sb = cst.tile([128, N_DC, DG], F32R, tag="wq")
            wk_sb = cst.tile([128, N_DC, DG], F32R, tag="wk")
            wv_sb = cst.tile([128, N_DC, DG], F32R, tag="wv")
            wo_sb = cst.tile([128, 2, D_MODEL], F32R, tag="wo")
            bq_sb = cst.tile([128, 2], F32, tag="bq")
            bk_sb = cst.tile([128, 2], F32, tag="bk")
            bv_sb = cst.tile([1, DG], F32R, tag="bv")
            mask_sb = cst.tile([128, 2, 2, QB], F32R, tag="mask")
            nc.sync.dma_start(wq_sb, wq.rearrange("(c p) m -> p c m", p=128))
            nc.sync.dma_start(wk_sb, wk.rearrange("(c p) m -> p c m", p=128))
            nc.sync.dma_start(wv_sb, wv.rearrange("(c p) m -> p c m", p=128))
            nc.sync.dma_start(wo_sb, wo.rearrange("(c p) m -> p c m", p=128))
            nc.sync.dma_start(bq_sb, bq[:, :])
            nc.sync.dma_start(bk_sb, bk[:, :])
            nc.sync.dma_start(bv_sb, bv[:, :])
            nc.sync.dma_start(mask_sb, masks[:, :, :, :])

            ones_f = cst.tile([128, HPC], F32, tag="ones_f")
            nc.vector.memset(ones_f, 1.0)
            ones1_f = cst.tile([1, 128], F32, tag="ones1_f")
            nc.vector.memset(ones1_f, 1.0)
            ones1 = cst.tile([1, 128], F32R, tag="ones1")
            nc.vector.tensor_copy(ones1, ones1_f)

            # ---- resident activation tensors ----
            # QT/KT: per head-pair t: [128 (2x64 dims), L]
            qt_sb = [cst.tile([128, L], F32R, tag=f"qt{t}", name=f"qt{t}")
                     for t in range(2)]
            kt_sb = [cst.tile([128, L], F32R, tag=f"kt{t}", name=f"kt{t}")
                     for t in range(2)]
            # OT: per head-pair t: [128 (2x64 dims), L] (normalized)
            ot_sb = [cst.tile([128, L], F32R, tag=f"ot{t}", name=f"ot{t}")
                     for t in range(2)]
            # V natural with ones column: per token tile: [128 tok, 4 heads, 65]
            v_sb = [cst.tile([128, HPC, HD + 1], F32R, tag=f"v{tt}", name=f"v{tt}")
                    for tt in range(N_TT)]

            def qkv_block(tb):
                """QKV projection for token block tb (512 tokens)."""
                xt_t = xtp.tile([128, N_DC, QB], F32R, tag="xt", name="xt_t")
                nc.sync.dma_start(
                    xt_t,
                    xt[:, tb * QB:(tb + 1) * QB].rearrange("(c p) t -> p c t", p=128),
                )
                # Q and K (transposed layout), per head-pair
                for t in range(2):
                    for which, w_sb, b_sb, dst in (
                        ("q", wq_sb, bq_sb, qt_sb),
                        ("k", wk_sb, bk_sb, kt_sb),
                    ):
                        acc = ps_mm.tile([128, QB], F32, tag="mm", name="acc")
                        for c in range(N_DC):
                            nc.tensor.matmul(
                                acc,
                                w_sb[:, c, 128 * t:128 * (t + 1)],
                                xt_t[:, c, :],
                                start=(c == 0), stop=(c == N_DC - 1),
                            )
                        nc.vector.tensor_scalar_add(
                            dst[t][:, tb * QB:(tb + 1) * QB], acc, b_sb[:, t:t + 1],
                        )
                # V (natural layout), per token tile
                for j in range(QB // 128):
                    tt = tb * (QB // 128) + j
                    vps = ps_mm.tile([128, DG], F32, tag="mm", name="vps")
                    for c in range(N_DC):
                        nc.tensor.matmul(
                            vps,
                            xt_t[:, c, j * 128:(j + 1) * 128],
                            wv_sb[:, c, :],
                            start=(c == 0), stop=False,
                        )
                    nc.tensor.matmul(vps, ones1, bv_sb, start=False, stop=True)
                    nc.vector.tensor_copy(
                        v_sb[tt][:, :, 0:HD],
                        vps.rearrange("p (h d) -> p h d", h=HPC),
                    )
                    nc.vector.tensor_copy(v_sb[tt][:, :, HD], ones_f)

            def attn_block(qb):
                """Attention for q-block qb, all 4 heads (2 pairs)."""
                n_kc = 4 * (qb + 1)           # causal: k-chunks 0..n_kc-1
                n_g = n_kc // 2               # groups of 2 chunks
                for t in range(2):
                    ot_p = {hp: ps_ot.tile([HD + 1, QB], F32, tag=f"otp{hp}",
                                           name=f"otp{hp}") for hp in range(2)}
                    for g in range(n_g):
                        st = [ps_st.tile([128, 2, QB], F32, tag=f"st{hp}",
                                         name=f"st{hp}") for hp in range(2)]
                        for c in range(2):
                            kc = 2 * g + c
                            for hp in range(2):
                                nc.tensor.matmul(
                                    st[hp][:, c, :],
                                    kt_sb[t][64 * hp:64 * (hp + 1),
                                             kc * 128:(kc + 1) * 128],
                                    qt_sb[t][64 * hp:64 * (hp + 1),
                                             qb * QB:(qb + 1) * QB],
                                    start=True, stop=True,
                                )
                        base = QB * qb - 256 * g
                        for hp in range(2):
                            p_t = ptp.tile([128, 2, QB], F32R, tag=f"pt{hp}",
                                           name=f"pt{hp}")
                            nc.scalar.activation(p_t, st[hp], AF.Exp)
                            if base < 255:   # diagonal group: base is 0 or -256
                                mi = (-base) // 256
                                nc.vector.tensor_mul(p_t, p_t, mask_sb[:, mi, :, :])
                            for c in range(2):
                                kc = 2 * g + c
                                nc.tensor.matmul(
                                    ot_p[hp],
                                    v_sb[kc][:, 2 * t + hp, 0:HD + 1],
                                    p_t[:, c, :],
                                    start=(kc == 0), stop=(kc == n_kc - 1),
                                )
                    # normalize pair t -> OT_sb
                    for hp in range(2):
                        rs = nrm.tile([1, QB], F32, tag="rs", name="rs")
                        nc.vector.reciprocal(rs, ot_p[hp][HD:HD + 1, :])
                        rbc = nrm.tile([64, QB], F32, tag="rbc", name="rbc")
                        nc.gpsimd.partition_broadcast(rbc, rs)
                        nc.vector.tensor_mul(
                            ot_sb[t][64 * hp:64 * (hp + 1), qb * QB:(qb + 1) * QB],
                            ot_p[hp][0:HD, :],
                            rbc,
                        )

            def outproj_block(qb):
                """Output projection for q-block qb."""
                for j in range(QB // 128):
                    tt = qb * (QB // 128) + j
                    ob = osb.tile([128, D_MODEL], F32, tag="ob", name="ob")
                    for dc in range(2):
                        ops = ps_mm.tile([128, 512], F32, tag="mm", name="ops")
                        for t in range(2):
                            nc.tensor.matmul(
                                ops,
                                ot_sb[t][:, tt * 128:(tt + 1) * 128],
                                wo_sb[:, t, dc * 512:(dc + 1) * 512],
                                start=(t == 0), stop=(t == 1),
                            )
                        nc.vector.tensor_copy(ob[:, dc * 512:(dc + 1) * 512], ops)
                    nc.sync.dma_start(out[tt * 128:(tt + 1) * 128, :], ob)

            # ---- emission order: pipeline QKV blocks with attention blocks ----
            qkv_block(0)
            qkv_block(1)
            attn_block(0)
            outproj_block(0)
            qkv_block(2)
            attn_block(1)
            outproj_block(1)
            qkv_block(3)
            attn_block(2)
            outproj_block(2)
            attn_block(3)
            outproj_block(3)

    nc.compile()
    return nc


_NC_CACHE = None


def _get_nc():
    global _NC_CACHE
    if _NC_CACHE is None:
        _NC_CACHE = _build()
    return _NC_CACHE


def _make_masks():
    p_ = np.arange(128)[:, None, None, None]
    i_ = np.arange(2)[None, :, None, None]
    c_ = np.arange(2)[None, None, :, None]
    q_ = np.arange(QB)[None, None, None, :]
    return np.ascontiguousarray(
        ((-256 * i_ + q_ - 128 * c_ - p_) >= 0).astype(np.float32))


def kernel(x, qkv_w, qkv_b, out_w, out_b, _trace=False):
    x = np.asarray(x, dtype=np.float32)
    qkv_w = np.asarray(qkv_w, dtype=np.float32)
    qkv_b = np.asarray(qkv_b, dtype=np.float32)
    out_w = np.asarray(out_w, dtype=np.float32)
    out_b = np.asarray(out_b, dtype=np.float32)

    scale = 1.0 / np.sqrt(HD)
    wq_full = qkv_w[0:D_MODEL] * scale          # [1024, 1024]
    wk_full = qkv_w[D_MODEL:2 * D_MODEL]
    wv_full = qkv_w[2 * D_MODEL:3 * D_MODEL]
    bq_full = qkv_b[0:D_MODEL] * scale
    bk_full = qkv_b[D_MODEL:2 * D_MODEL]
    bv_full = qkv_b[2 * D_MODEL:3 * D_MODEL]

    masks = _make_masks()
    in_maps = []
    for core in range(8):
        b, g = core // 4, core % 4
        sl = slice(DG * g, DG * (g + 1))
        in_maps.append({
            "xt": np.ascontiguousarray(x[b].T),
            "wq": np.ascontiguousarray(wq_full[sl].T),
            "wk": np.ascontiguousarray(wk_full[sl].T),
            "wv": np.ascontiguousarray(wv_full[sl].T),
            "wo": np.ascontiguousarray(out_w[:, sl].T),
            "bq": np.ascontiguousarray(bq_full[sl].reshape(2, 128).T),
            "bk": np.ascontiguousarray(bk_full[sl].reshape(2, 128).T),
            "bv": np.ascontiguousarray(bv_full[sl].reshape(1, DG)),
            "masks": masks,
        })

    nc = _get_nc()
    res = run_bass_kernel_spmd(nc, in_maps, core_ids=list(range(8)),
                               trace=_trace)

    final = np.zeros((B, L, D_MODEL), dtype=np.float32)
    for core in range(8):
        b = core // 4
        final[b] += res.results[core]["out"]
    final += out_b[None, None, :]

    if _trace:
        kernel.last_results = res
    return final


# revision 11
# speedup vs baseline: 1.0466x; 1.0466x over previous
"""Causal self-attention Trainium2 kernel.

Problem: B=2, L=2048, D=1024, 16 heads (hd=64), fp32.

Sharding (8 cores): core = (batch b in {0,1}) x (head-group g in {0..3} of 4
heads). Each core:
  - reads x[b]^T  [1024, 2048]
  - QKV projection for its 4 heads (fp32r matmuls, full PE rate at N>=256)
  - causal attention in transposed layout:
      S^T[k, q] = K^T(lhsT) x Q^T(rhs), two heads row-packed per matmul (K=64)
      P^T = exp(S^T)  (ACT), causal mask via 0/1 mask multiply (DVE)
      O^T[d, q] accumulated as [V | ones]^T(lhsT) x P^T(rhs) -> row 64 = rowsum
      normalize: reciprocal (DVE) -> partition_broadcast (GPSIMD) -> mul (DVE)
  - output projection partial: OUT[tok, :] = O^T-chunks(lhsT) x Wo^T(rhs)
Host: sums the 4 head-group partials per batch, adds out_b.

All matmul operands are typed float32r (TF32-like, ~1.5e-4 rel err/matmul,
full 1 cycle/row PE rate at N>=256 vs 4 cycles/row for fp32).
"""
import os
import numpy as np

import concourse.bass as bass
import concourse.mybir as mybir
import concourse.tile as tile
from concourse import bacc
from concourse.bass_utils import run_bass_kernel_spmd

F32 = mybir.dt.float32
F32R = mybir.dt.float32r
AF = mybir.ActivationFunctionType

D_MODEL = 1024
N_HEADS = 16
HD = 64
B = 2
L = 2048                      # tokens per batch
HPC = 4                       # heads per core
DG = HPC * HD                 # 256 dims per core's head group
QB = 512                      # q-block width
N_QB = L // QB                # 4
N_KC = L // 128               # 16 k-chunks of 128 tokens
N_DC = D_MODEL // 128         # 8 d_model chunks
N_TT = L // 128               # 16 token tiles


def _build():
    nc = bacc.Bacc("TRN2", target_bir_lowering=False)

    xt = nc.dram_tensor("xt", [D_MODEL, L], F32R, kind="ExternalInput")
    wq = nc.dram_tensor("wq", [D_MODEL, DG], F32R, kind="ExternalInput")
    wk = nc.dram_tensor("wk", [D_MODEL, DG], F32R, kind="ExternalInput")
    wv = nc.dram_tensor("wv", [D_MODEL, DG], F32R, kind="ExternalInput")
    wo = nc.dram_tensor("wo", [DG, D_MODEL], F32R, kind="ExternalInput")
    bq = nc.dram_tensor("bq", [128, 2], F32, kind="ExternalInput")
    bk = nc.dram_tensor("bk", [128, 2], F32, kind="ExternalInput")
    bv = nc.dram_tensor("bv", [1, DG], F32R, kind="ExternalInput")
    # masks[p, i, c, q] = 1 iff -256*i + q - 128*c - p >= 0  (i in {0,1})
    masks = nc.dram_tensor("masks", [128, 2, 2, QB], F32R, kind="ExternalInput")
    out = nc.dram_tensor("out", [L, D_MODEL], F32, kind="ExternalOutput")

    with tile.TileContext(nc) as tc:
        with (
            tc.tile_pool(name="cst", bufs=1) as cst,
            tc.tile_pool(name="xtp", bufs=3) as xtp,
            tc.tile_pool(name="ptp", bufs=3) as ptp,
            tc.tile_pool(name="nrm", bufs=2) as nrm,
            tc.tile_pool(name="osb", bufs=2) as osb,
            tc.tile_pool(name="ps_st", bufs=1, space="PSUM") as ps_st,
            tc.tile_pool(name="ps_ot", bufs=1, space="PSUM") as ps_ot,
            tc.tile_pool(name="ps_mm", bufs=2, space="PSUM") as ps_mm,
        ):
            # ---- constants / weights ----
            wq_sb = cst.tile([128, N_DC, DG], F32R, tag="wq")
            wk_sb = cst.tile([128, N_DC, DG], F32R, tag="wk")
            wv_sb = cst.tile([128, N_DC, DG], F32R, tag="wv")
            wo_sb = cst.tile([128, 2, D_MODEL], F32R, tag="wo")
            bq_sb = cst.tile([128, 2], F32, tag="bq")
            bk_sb = cst.tile([128, 2], F32, tag="bk")
            bv_sb = cst.tile([1, DG], F32R, tag="bv")
            mask_sb = cst.tile([128, 2, 2, QB], F32R, tag="mask")
            # DMA queues: SP ring carries xt blocks + outputs (critical path),
            # ACT ring carries QKV weights, SWDGE carries late bulk (masks/wo)
            nc.scalar.dma_start(wq_sb, wq.rearrange("(c p) m -> p c m", p=128))
            nc.scalar.dma_start(bq_sb, bq[:, :])
            nc.scalar.dma_start(wk_sb, wk.rearrange("(c p) m -> p c m", p=128))
            nc.scalar.dma_start(bk_sb, bk[:, :])
            nc.scalar.dma_start(wv_sb, wv.rearrange("(c p) m -> p c m", p=128))
            nc.scalar.dma_start(bv_sb, bv[:, :])
            nc.gpsimd.dma_start(mask_sb, masks[:, :, :, :])
            nc.gpsimd.dma_start(wo_sb, wo.rearrange("(c p) m -> p c m", p=128))

            ones_f = cst.tile([128, HPC], F32, tag="ones_f")
            nc.vector.memset(ones_f, 1.0)
            ones1_f = cst.tile([1, 128], F32, tag="ones1_f")
            nc.vector.memset(ones1_f, 1.0)
            ones1 = cst.tile([1, 128], F32R, tag="ones1")
            nc.vector.tensor_copy(ones1, ones1_f)

            # ---- resident activation tensors ----
            # QT/KT: per head-pair t: [128 (2x64 dims), L]
            qt_sb = [cst.tile([128, L], F32R, tag=f"qt{t}", name=f"qt{t}")
                     for t in range(2)]
            kt_sb = [cst.tile([128, L], F32R, tag=f"kt{t}", name=f"kt{t}")
                     for t in range(2)]
            # OT: per head-pair t: [128 (2x64 dims), L] (normalized)
            ot_sb = [cst.tile([128, L], F32R, tag=f"ot{t}", name=f"ot{t}")
                     for t in range(2)]
            # V natural with ones column: per token tile: [128 tok, 4 heads, 65]
            v_sb = [cst.tile([128, HPC, HD + 1], F32R, tag=f"v{tt}", name=f"v{tt}")
                    for tt in range(N_TT)]

            def load_xt(tb):
                xt_t = xtp.tile([128, N_DC, QB], F32R, tag="xt", name="xt_t")
                nc.sync.dma_start(
                    xt_t,
                    xt[:, tb * QB:(tb + 1) * QB].rearrange("(c p) t -> p c t", p=128),
                )
                return xt_t

            def qkv_block(tb, xt_t):
                """QKV projection for token block tb (512 tokens)."""
                # Q and K (transposed layout), per head-pair
                for t in range(2):
                    for which, w_sb, b_sb, dst in (
                        ("q", wq_sb, bq_sb, qt_sb),
                        ("k", wk_sb, bk_sb, kt_sb),
                    ):
                        acc = ps_mm.tile([128, QB], F32, tag="mm", name="acc")
                        for c in range(N_DC):
                            nc.tensor.matmul(
                                acc,
                                w_sb[:, c, 128 * t:128 * (t + 1)],
                                xt_t[:, c, :],
                                start=(c == 0), stop=(c == N_DC - 1),
                            )
                        nc.vector.tensor_scalar_add(
                            dst[t][:, tb * QB:(tb + 1) * QB], acc, b_sb[:, t:t + 1],
                        )
                # V (natural layout), per token tile
                for j in range(QB // 128):
                    tt = tb * (QB // 128) + j
                    vps = ps_mm.tile([128, DG], F32, tag="mm", name="vps")
                    for c in range(N_DC):
                        nc.tensor.matmul(
                            vps,
                            xt_t[:, c, j * 128:(j + 1) * 128],
                            wv_sb[:, c, :],
                            start=(c == 0), stop=False,
                        )
                    nc.tensor.matmul(vps, ones1, bv_sb, start=False, stop=True)
                    nc.vector.tensor_copy(
                        v_sb[tt][:, :, 0:HD],
                        vps.rearrange("p (h d) -> p h d", h=HPC),
                    )
                    nc.vector.tensor_copy(v_sb[tt][:, :, HD], ones_f)

            def attn_block(qb):
                """Attention for q-block qb, all 4 heads (2 pairs)."""
                n_kc = 4 * (qb + 1)           # causal: k-chunks 0..n_kc-1
                n_g = n_kc // 2               # groups of 2 chunks
                for t in range(2):
                    ot_p = {hp: ps_ot.tile([HD + 1, QB], F32, tag=f"otp{hp}",
                                           name=f"otp{hp}") for hp in range(2)}
                    for g in range(n_g):
                        st = [ps_st.tile([128, 2, QB], F32, tag=f"st{hp}",
                                         name=f"st{hp}") for hp in range(2)]
                        for c in range(2):
                            kc = 2 * g + c
                            for hp in range(2):
                                nc.tensor.matmul(
                                    st[hp][:, c, :],
                                    kt_sb[t][64 * hp:64 * (hp + 1),
                                             kc * 128:(kc + 1) * 128],
                                    qt_sb[t][64 * hp:64 * (hp + 1),
                                             qb * QB:(qb + 1) * QB],
                                    start=True, stop=True,
                                )
                        base = QB * qb - 256 * g
                        for hp in range(2):
                            p_t = ptp.tile([128, 2, QB], F32R, tag=f"pt{hp}",
                                           name=f"pt{hp}")
                            nc.scalar.activation(p_t, st[hp], AF.Exp)
                            if base < 255:   # diagonal group: base is 0 or -256
                                mi = (-base) // 256
                                nc.vector.tensor_mul(p_t, p_t, mask_sb[:, mi, :, :])
                            for c in range(2):
                                kc = 2 * g + c
                                nc.tensor.matmul(
                                    ot_p[hp],
                                    v_sb[kc][:, 2 * t + hp, 0:HD + 1],
                                    p_t[:, c, :],
                                    start=(kc == 0), stop=(kc == n_kc - 1),
                                )
                    # normalize pair t -> OT_sb: fast recip, broadcast, multiply
                    for hp in range(2):
                        rs = nrm.tile([1, QB], F32, tag="rs", name="rs")
                        nc.vector.reciprocal(rs, ot_p[hp][HD:HD + 1, :])
                        rbc = nrm.tile([64, QB], F32, tag="rbc", name="rbc")
                        nc.gpsimd.partition_broadcast(rbc, rs)
                        nc.vector.tensor_mul(
                            ot_sb[t][64 * hp:64 * (hp + 1), qb * QB:(qb + 1) * QB],
                            ot_p[hp][0:HD, :],
                            rbc,
                        )

            def outproj_block(qb):
                """Output projection for q-block qb."""
                for j in range(QB // 128):
                    tt = qb * (QB // 128) + j
                    ob = osb.tile([128, D_MODEL], F32, tag="ob", name="ob")
                    for dc in range(2):
                        ops = ps_mm.tile([128, 512], F32, tag="mm", name="ops")
                        for t in range(2):
                            nc.tensor.matmul(
                                ops,
                                ot_sb[t][:, tt * 128:(tt + 1) * 128],
                                wo_sb[:, t, dc * 512:(dc + 1) * 512],
                                start=(t == 0), stop=(t == 1),
                            )
                        nc.vector.tensor_copy(ob[:, dc * 512:(dc + 1) * 512], ops)
                    nc.sync.dma_start(out[tt * 128:(tt + 1) * 128, :], ob)

            # ---- emission order: pipeline QKV blocks with attention blocks ----
            xt0 = load_xt(0)
            xt1 = load_xt(1)
            qkv_block(0, xt0)
            xt2 = load_xt(2)
            qkv_block(1, xt1)
            attn_block(0)
            outproj_block(0)
            xt3 = load_xt(3)
            qkv_block(2, xt2)
            attn_block(1)
            outproj_block(1)
            qkv_block(3, xt3)
            attn_block(2)
            outproj_block(2)
            attn_block(3)
            outproj_block(3)

    nc.compile()
    return nc


_NC_CACHE = None


def _get_nc():
    global _NC_CACHE
    if _NC_CACHE is None:
        _NC_CACHE = _build()
    return _NC_CACHE


def _make_masks():
    p_ = np.arange(128)[:, None, None, None]
    i_ = np.arange(2)[None, :, None, None]
    c_ = np.arange(2)[None, None, :, None]
    q_ = np.arange(QB)[None, None, None, :]
    return np.ascontiguousarray(
        ((-256 * i_ + q_ - 128 * c_ - p_) >= 0).astype(np.float32))


def kernel(x, qkv_w, qkv_b, out_w, out_b, _trace=False):
    x = np.asarray(x, dtype=np.float32)
    qkv_w = np.asarray(qkv_w, dtype=np.float32)
    qkv_b = np.asarray(qkv_b, dtype=np.float32)
    out_w = np.asarray(out_w, dtype=np.float32)
    out_b = np.asarray(out_b, dtype=np.float32)

    scale = 1.0 / np.sqrt(HD)
    wq_full = qkv_w[0:D_MODEL] * scale          # [1024, 1024]
    wk_full = qkv_w[D_MODEL:2 * D_MODEL]
    wv_full = qkv_w[2 * D_MODEL:3 * D_MODEL]
    bq_full = qkv_b[0:D_MODEL] * scale
    bk_full = qkv_b[D_MODEL:2 * D_MODEL]
    bv_full = qkv_b[2 * D_MODEL:3 * D_MODEL]

    masks = _make_masks()
    in_maps = []
    for core in range(8):
        b, g = core // 4, core % 4
        sl = slice(DG * g, DG * (g + 1))
        in_maps.append({
            "xt": np.ascontiguousarray(x[b].T),
            "wq": np.ascontiguousarray(wq_full[sl].T),
            "wk": np.ascontiguousarray(wk_full[sl].T),
            "wv": np.ascontiguousarray(wv_full[sl].T),
            "wo": np.ascontiguousarray(out_w[:, sl].T),
            "bq": np.ascontiguousarray(bq_full[sl].reshape(2, 128).T),
            "bk": np.ascontiguousarray(bk_full[sl].reshape(2, 128).T),
            "bv": np.ascontiguousarray(bv_full[sl].reshape(1, DG)),
            "masks": masks,
        })

    nc = _get_nc()
    res = run_bass_kernel_spmd(nc, in_maps, core_ids=list(range(8)),
                               trace=_trace)

    final = np.zeros((B, L, D_MODEL), dtype=np.float32)
    for core in range(8):
        b = core // 4
        final[b] += res.results[core]["out"]
    final += out_b[None, None, :]

    if _trace:
        kernel.last_results = res
    return final
